# revision 1
# baseline (speedup 1.0000x reference)
"""Trainium2 Bass kernel for causal multi-head attention (GPT-style block).

Reference computation (fp32):
    qkv = x @ w_attn + b_attn          # [B,T,3C]
    q,k,v per head (12 heads, d=64)
    att = softmax(causal(q k^T / 8))
    y   = att @ v
    out = y @ w_proj + b_proj

Sharding: 8 cores = 2 batches x 4 head-groups (3 heads each).
Tensor-parallel over heads: each core takes its 3 heads' columns of
w_attn and rows of w_proj, computes a partial [T, C] output; the host
sums the 4 head-group partials per batch (the "all-reduce") + b_proj.

Device kernel (per core). Matmul dtype is bf16 by default (f32r via
KERNEL_MM_DTYPE=f32r). PSUM accumulation is always fp32.

  1. x^T arrives pre-transposed [768, T] from host sharding; 6
     contiguous DMAs into SBUF.
  2. qkv^T = w^T x^T in 5 M-blocks laid out so each head's Q and K
     share a partition base: b0=[Q0|Q1] b1=[K0|K1] b2=[VT0|VT1]
     b3=[Q2|V2T] b4=[K2|--]  ([128,T] SBUF tiles, | = partition 64)
  3. V^T -> V k-tiles [128, 3*65] (64 V cols + ones col per head; the
     ones column turns the AV matmul into a fused softmax-sum).
     bf16: DRAM roundtrip + transposing DMAs; f32r: PE transposes.
  4. per q-chunk (512) per k-tile (128) per head:
       S^T[k,q] = K-slice.T @ Q    (PE; causal-sliced N in {512,384,256})
       + causal mask (-1e9) on the diagonal strip (DVE)
       eS = exp(0.125 * S^T)        (ACT)
       y^T[65,q] += [V|1].T @ eS    (PE, row 64 = softmax sum l)
     then l -> partition-broadcast via K=1 f32r PE matmul,
     rb = 1/l (DVE reciprocal_approx_fast), y^T *= rb (DVE)
  5. out[q,768] = y^T.T @ w_proj_rows  (PE), DMA out
"""

import os
import sys

import numpy as np

sys.path.insert(0, "/opt/trn_rl_repo")

from concourse import bacc, bass, mybir  # noqa: E402
from concourse import bass_utils  # noqa: E402
from concourse.tile import TileContext  # noqa: E402

P = 128
T = 2048
CDIM = 768
DHEAD = 64
HPC = 3  # heads per core
N_CORES = 8
FTOT = 3 * HPC * DHEAD  # 576 qkv features per core
QW = 512  # q-chunk width
NQC = T // QW  # 4
NTT = T // P  # 16 token tiles
NCT = CDIM // P  # 6 contraction tiles
FBLK = [128, 128, 128, 128, 64]  # M-block widths (sum=576)
SCALE = 1.0 / np.sqrt(DHEAD)
NEG = -1.0e9

DT = mybir.dt.float32
DTR = mybir.dt.float32r
MM_DTYPE = os.environ.get("KERNEL_MM_DTYPE", "bf16")
DTM = mybir.dt.bfloat16 if MM_DTYPE == "bf16" else DTR


def build_nc():
    from contextlib import ExitStack

    bf16 = MM_DTYPE == "bf16"
    nc = bacc.Bacc("TRN2", target_bir_lowering=False, debug=False)
    x_d = nc.dram_tensor("x", [CDIM, T], DTM, kind="ExternalInput")
    w_d = nc.dram_tensor("wqkv", [CDIM, FTOT], DTM, kind="ExternalInput")
    b_d = nc.dram_tensor("bqkv", [5, P], DT, kind="ExternalInput")
    wp_d = nc.dram_tensor("wproj", [HPC * DHEAD, CDIM], DTM, kind="ExternalInput")
    m_d = nc.dram_tensor("mask", [P, 2 * P], DT, kind="ExternalInput")
    i_d = nc.dram_tensor("ident", [P, P], DTM, kind="ExternalInput")
    on_d = nc.dram_tensor("ones", [P, 64], DTM, kind="ExternalInput")
    onr_d = nc.dram_tensor("ones_r", [1, 64], DTR, kind="ExternalInput")
    o_d = nc.dram_tensor("out", [T, CDIM], DT, kind="ExternalOutput")

    EXP = mybir.ActivationFunctionType.Exp

    with TileContext(nc) as tc, ExitStack() as ctx:
        const = ctx.enter_context(tc.tile_pool(name="const", bufs=1))
        big = ctx.enter_context(tc.tile_pool(name="big", bufs=1))
        work = ctx.enter_context(tc.tile_pool(name="work", bufs=4))

        # ---- 1. x -> x^T on the sync ring (needed first); other consts
        # ---- on the ACT hwdge ring ----
        # x comes in pre-transposed [CDIM, T] (host does the transpose as
        # part of sharding) -> straight contiguous DMAs, no PE transposes.
        # One tile per (c, t-chunk) so the first qkv chain only waits on
        # the 6 chunk-0 DMAs, not the whole 3MB.
        xTt = [
            [
                big.tile([P, QW], DTM, tag=f"xT{c}_{t}", name=f"xT{c}_{t}")
                for t in range(NQC)
            ]
            for c in range(NCT)
        ]
        for t in range(NQC):
            for c in range(NCT):
                nc.sync.dma_start(
                    xTt[c][t][:],
                    x_d[c * P : (c + 1) * P, t * QW : (t + 1) * QW],
                )
        id_t = const.tile([P, P], DTM, tag="ident")
        nc.sync.dma_start(id_t[:], i_d[:])
        ps = ctx.enter_context(
            tc.tile_pool(name="ps", bufs=1, space=bass.MemorySpace.PSUM)
        )

        wt = []
        for c in range(NCT):
            w_c = const.tile([P, FTOT], DTM, tag=f"w{c}")
            nc.scalar.dma_start(w_c[:], w_d[c * P : (c + 1) * P, :])
            wt.append(w_c)
        bias_t = const.tile([P, 5], DT, tag="bias")
        nc.scalar.dma_start(bias_t[:], b_d[:].rearrange("b p -> p b"))
        mask_t = const.tile([P, 2 * P], DT, tag="mask")
        nc.scalar.dma_start(mask_t[:], m_d[:])
        wp0 = const.tile([P, CDIM], DTM, tag="wp0")
        nc.scalar.dma_start(wp0[:], wp_d[0:P, :])
        wp1 = const.tile([64, CDIM], DTM, tag="wp1")
        nc.scalar.dma_start(wp1[:], wp_d[P : P + 64, :])
        ones_r = const.tile([65, 64], DTR, tag="ones_r")
        nc.scalar.dma_start(ones_r[64:65, :], onr_d[:])

        # ---- 2. qkv^T in 5 M-blocks ----
        blk = [big.tile([P, T], DTM, tag=f"blk{i}", name=f"blk{i}") for i in range(5)]
        for tc_i in range(NQC):
            cols = slice(tc_i * QW, (tc_i + 1) * QW)
            for bi in range(5):
                w_off = sum(FBLK[:bi])
                psq = ps.tile([P, QW], DT, tag="misc", bufs=2)
                out_ap = psq[0 : FBLK[bi], :]
                for c in range(NCT):
                    nc.tensor.matmul(
                        out_ap,
                        wt[c][:, w_off : w_off + FBLK[bi]],
                        xTt[c][tc_i][:],
                        start=(c == 0),
                        stop=(c == NCT - 1),
                    )
                nc.vector.tensor_scalar_add(
                    blk[bi][0 : FBLK[bi], cols],
                    out_ap,
                    bias_t[0 : FBLK[bi], bi : bi + 1],
                )

        # ---- 3. V^T -> V (k-tiles with a ones column per head) ----
        # V^T head slices: h0: blk2[0:64], h1: blk2[64:128], h2: blk3[64:128]
        vsrc = [(2, 0), (2, 64), (3, 64)]
        vk = [
            big.tile([P, 3 * 65], DTM, tag=f"vk{j}", name=f"vk{j}")
            for j in range(NTT)
        ]
        for j in range(NTT):
            nc.scalar.dma_start(
                vk[j][:].rearrange("p (h c) -> p h c", c=65)[:, :, 64:65],
                on_d[:, 0:3].unsqueeze(2),
            )
        for j in range(NTT):
            for h in range(HPC):
                sb, r0 = vsrc[h]
                pst = ps.tile([P, 64], DT, tag="misc", bufs=2)
                if bf16:
                    nc.tensor.matmul(
                        pst[:],
                        blk[sb][r0 : r0 + 64, j * P : (j + 1) * P],
                        id_t[r0 : r0 + 64, r0 : r0 + 64],
                        start=True,
                        stop=True,
                    )
                    nc.vector.tensor_copy(vk[j][:, 65 * h : 65 * h + 64], pst[:])
                else:
                    nc.tensor.matmul(
                        pst[:].bitcast(DTM),
                        blk[sb][r0 : r0 + 64, j * P : (j + 1) * P],
                        id_t[r0 : r0 + 64, r0 : r0 + 64],
                        is_transpose=True,
                    )
                    nc.vector.tensor_copy(
                        vk[j][:, 65 * h : 65 * h + 64], pst[:].bitcast(DTM)
                    )

        # ---- 4. attention ----
        # Q/K head slices (block idx, row base)
        qsrc = [(0, 0), (0, 64), (3, 0)]
        ksrc = [(1, 0), (1, 64), (4, 0)]
        yT0 = big.tile([P, T], DTM, tag="yT0")  # rows: h0 | h1
        yT1 = big.tile([64, T], DTM, tag="yT1")  # h2
        for qc in range(NQC):
            qcols = slice(qc * QW, (qc + 1) * QW)
            nk = 4 * (qc + 1)
            psy = [
                ps.tile([65, QW], DT, tag=f"y{h}", bufs=1, name=f"psy{h}")
                for h in range(HPC)
            ]
            for j in range(nk):
                # diagonal tile m: cols < 128*m are fully masked; compute
                # only [cs:512] (cs capped at 256 so N stays >= 256).
                m = j - 4 * qc
                cs = 0 if m < 1 else m * P
                ssl = slice(cs, QW)
                qsl_g = slice(qc * QW + cs, (qc + 1) * QW)
                for h in range(HPC):
                    qb, qr = qsrc[h]
                    kb, kr = ksrc[h]
                    pss = ps.tile([P, QW], DT, tag="s", bufs=3, name="pss")
                    nc.tensor.matmul(
                        pss[:, ssl],
                        blk[kb][kr : kr + 64, j * P : (j + 1) * P],
                        blk[qb][qr : qr + 64, qsl_g],
                        start=True,
                        stop=True,
                    )
                    if m >= 0:
                        msl = slice(m * P, (m + 1) * P)
                        nc.vector.tensor_add(
                            pss[:, msl], pss[:, msl], mask_t[:, P : 2 * P]
                        )
                    es = work.tile([P, QW], DTM, tag="es", bufs=6, name="es")
                    nc.scalar.activation(
                        es[:, ssl], pss[:, ssl], EXP, scale=float(SCALE)
                    )
                    nc.tensor.matmul(
                        psy[h][:, ssl],
                        vk[j][:, 65 * h : 65 * h + 65],
                        es[:, ssl],
                        start=(j == 0),
                        stop=(j == nk - 1),
                    )
            for h in range(HPC):
                lrow = work.tile([65, QW], DTR, tag="l", bufs=2)
                nc.vector.tensor_copy(lrow[64:65, :], psy[h][64:65, :])
                rbp = ps.tile([64, QW], DT, tag="misc", bufs=2)
                nc.tensor.matmul(
                    rbp[:], ones_r[64:65, :], lrow[64:65, :], start=True, stop=True
                )
                rb = work.tile([64, QW], DT, tag="rb", bufs=2)
                nc.vector.reciprocal_approx_fast(out=rb[:], in_=rbp[:])
                if h == 0:
                    nc.vector.tensor_mul(yT0[0:64, qcols], psy[h][0:64, :], rb[:])
                elif h == 2:
                    nc.vector.tensor_mul(yT1[0:64, qcols], psy[h][0:64, :], rb[:])
                else:
                    tmp = work.tile([64, QW], DTM, tag="ytmp", bufs=2)
                    nc.vector.tensor_mul(tmp[:], psy[h][0:64, :], rb[:])
                    nc.scalar.dma_start(yT0[64:P, qcols], tmp[:])

            # ---- 5. projection for this q-chunk (overlaps next chunk) ----
            for qt in range(4 * qc, 4 * qc + 4):
                qsl = slice(qt * P, (qt + 1) * P)
                ost = work.tile([P, CDIM], DT, tag="ost", bufs=3)
                for n in range(2):
                    nsl = slice(n * 384, (n + 1) * 384)
                    pso = ps.tile([P, 384], DT, tag="misc", bufs=2)
                    nc.tensor.matmul(
                        pso[:], yT0[:, qsl], wp0[:, nsl], start=True, stop=False
                    )
                    nc.tensor.matmul(
                        pso[:], yT1[:, qsl], wp1[:, nsl], start=False, stop=True
                    )
                    nc.vector.tensor_copy(ost[:, nsl], pso[:])
                nc.sync.dma_start(o_d[qsl, :], ost[:])



    nc.compile()
    return nc


_NC_CACHE = None


def _get_nc():
    global _NC_CACHE
    if _NC_CACHE is None:
        _NC_CACHE = build_nc()
    return _NC_CACHE


def _host_inputs(x, w_attn, b_attn, w_proj):
    """Per-core input dicts. Core c = batch (c//4), head-group (c%4)."""
    import ml_dtypes

    npm = ml_dtypes.bfloat16 if MM_DTYPE == "bf16" else np.float32
    x = np.ascontiguousarray(np.asarray(x, dtype=np.float32))
    w_attn = np.asarray(w_attn, dtype=np.float32)
    b_attn = np.asarray(b_attn, dtype=np.float32)
    w_proj = np.asarray(w_proj, dtype=np.float32)

    ident = np.eye(P, dtype=np.float32)
    # mask tile [128, 256]: cols 0-127 all NEG (fully-masked strip for the
    # m=3 case), cols 128-255 the p>f triangle used by every diagonal block
    pp, ff = np.meshgrid(np.arange(P), np.arange(P), indexing="ij")
    tri = np.where(pp > ff, np.float32(NEG), np.float32(0.0))
    mask = np.concatenate(
        [np.full((P, P), NEG, dtype=np.float32), tri], axis=1
    ).astype(np.float32)

    in_maps = []
    for core in range(N_CORES):
        b, hg = divmod(core, 4)
        hs = 3 * hg  # first head of this core
        # column ranges in the 2304-wide qkv dim: q at 0, k at 768, v at 1536
        q0, k0, v0 = 64 * hs, CDIM + 64 * hs, 2 * CDIM + 64 * hs
        # M-blocks: b0=[Q0|Q1] b1=[K0|K1] b2=[V0|V1] b3=[Q2|V2] b4=[K2]
        wqkv = np.concatenate(
            [
                w_attn[:, q0 : q0 + 128],
                w_attn[:, k0 : k0 + 128],
                w_attn[:, v0 : v0 + 128],
                w_attn[:, q0 + 128 : q0 + 192],
                w_attn[:, v0 + 128 : v0 + 192],
                w_attn[:, k0 + 128 : k0 + 192],
            ],
            axis=1,
        )
        bqkv = np.zeros((5, P), dtype=np.float32)
        bqkv[0] = b_attn[q0 : q0 + 128]
        bqkv[1] = b_attn[k0 : k0 + 128]
        bqkv[2] = b_attn[v0 : v0 + 128]
        bqkv[3, 0:64] = b_attn[q0 + 128 : q0 + 192]
        bqkv[3, 64:128] = b_attn[v0 + 128 : v0 + 192]
        bqkv[4, 0:64] = b_attn[k0 + 128 : k0 + 192]
        wproj = np.ascontiguousarray(w_proj[64 * hs : 64 * hs + 192, :])
        in_maps.append(
            {
                "x": np.ascontiguousarray(x[b].T.astype(npm)),
                "wqkv": np.ascontiguousarray(wqkv.astype(npm)),
                "bqkv": bqkv,
                "wproj": wproj.astype(npm),
                "mask": mask,
                "ident": ident.astype(npm),
                "ones": np.ones((P, 64), dtype=npm),
                "ones_r": np.ones((1, 64), dtype=np.float32),
            }
        )
    return in_maps


def run(x, w_attn, b_attn, w_proj, b_proj, trace=False):
    nc = _get_nc()
    in_maps = _host_inputs(x, w_attn, b_attn, w_proj)
    res = bass_utils.run_bass_kernel_spmd(
        nc, in_maps, core_ids=list(range(N_CORES)), trace=trace
    )
    B = 2
    out = np.zeros((B, T, CDIM), dtype=np.float64)
    for core in range(N_CORES):
        out[core // 4] += res.results[core]["out"].astype(np.float64)
    out += np.asarray(b_proj, dtype=np.float64)[None, None, :]
    return out.astype(np.float32), res


def kernel(x, w_attn, b_attn, w_proj, b_proj):
    out, _ = run(x, w_attn, b_attn, w_proj, b_proj, trace=False)
    return out



# revision 2
# speedup vs baseline: 1.0019x; 1.0019x over previous
"""Trainium2 Bass kernel for causal multi-head attention (GPT-style block), v2.

Reference computation (fp32):
    qkv = x @ w_attn + b_attn          # [B,T,3C]
    q,k,v per head (12 heads, d=64)
    att = softmax(causal(q k^T / 8))
    y   = att @ v
    out = y @ w_proj + b_proj

Sharding: 8 cores = 2 batches x 4 head-groups (3 heads each).
Host sums the 4 head-group partials per batch + b_proj_eff where
b_proj_eff = b_proj + b_v @ w_proj  (V-bias folded out exactly since
softmax rows sum to 1).

Device kernel (per core), bf16 matmuls, fp32 PSUM:
  per t-chunk tc (512 tokens):
  1. QK^T blocks: b0=[Q0|Q1] b1=[K0|K1] b2=[Q2|K2]  ([128,512] psum,
     3 M-passes x 6 k-tiles), + per-partition bias -> SBUF bf16.
     q2dup: DMA copy of Q2 to partitions 64-127 (so h2's S matmul has
     lhsT/rhs at matching row group 64).
  2. V direct in [t, d] layout: per k-tile j, 6 matmuls with the x
     chunk slice as *stationary* -> psV[128t, 192d] -> vk[j] [128, 3*65]
     (64 V cols + ones col per head; ones turns AV into fused softmax-sum).
  3. attention qc=tc: per k-tile j (0..4tc+3):
       S^T[k,q] for h0 (rows 0-63) || h1, h2 (rows 64-127) -> one
         3-bank psum tile [128, 3*512]
       one tri-ACTIVATE exp(0.125 * S^T) over all 3 heads -> es bf16
       causal diagonal handled post-exp: 0/1 mask multiply on GpSimd
       AV: psy[h][65, q] += [V|1].T @ es  (row 64 = softmax sum l)
     then per h: l -> partition-broadcast via K=1 f32r matmul,
     rb = 1/l (DVE reciprocal_approx_fast), y^T = psy * rb (DVE)
  4. projection: out[q,768] = y^T.T @ w_proj_rows (PE), bf16 copy, DMA.
"""

import sys

import numpy as np

sys.path.insert(0, "/opt/trn_rl_repo")

from concourse import bacc, bass, mybir  # noqa: E402
from concourse import bass_utils  # noqa: E402
from concourse.tile import TileContext  # noqa: E402

P = 128
T = 2048
CDIM = 768
DHEAD = 64
HPC = 3  # heads per core
N_CORES = 8
QW = 512  # q-chunk width
NQC = T // QW  # 4
NTT = T // P  # 16 token tiles
NCT = CDIM // P  # 6 contraction tiles
SCALE = 1.0 / np.sqrt(DHEAD)

DT = mybir.dt.float32
DTR = mybir.dt.float32r
DTM = mybir.dt.bfloat16


def build_nc():
    from contextlib import ExitStack

    nc = bacc.Bacc("TRN2", target_bir_lowering=False, debug=False)
    x_d = nc.dram_tensor("x", [CDIM, T], DTM, kind="ExternalInput")
    wqk_d = nc.dram_tensor("wqk", [CDIM, 3 * P], DTM, kind="ExternalInput")
    wv_d = nc.dram_tensor("wv", [CDIM, HPC * DHEAD], DTM, kind="ExternalInput")
    b_d = nc.dram_tensor("bqk", [3, P], DT, kind="ExternalInput")
    wp_d = nc.dram_tensor("wproj", [HPC * DHEAD, CDIM], DTM, kind="ExternalInput")
    m_d = nc.dram_tensor("tri01", [P, HPC * P], DTM, kind="ExternalInput")
    on_d = nc.dram_tensor("ones", [P, HPC], DTM, kind="ExternalInput")
    onr_d = nc.dram_tensor("ones_r", [1, 64], DTR, kind="ExternalInput")
    o_d = nc.dram_tensor("out", [T, CDIM], DTM, kind="ExternalOutput")

    EXP = mybir.ActivationFunctionType.Exp

    with TileContext(nc) as tc, ExitStack() as ctx:
        const = ctx.enter_context(tc.tile_pool(name="const", bufs=1))
        big = ctx.enter_context(tc.tile_pool(name="big", bufs=1))
        work = ctx.enter_context(tc.tile_pool(name="work", bufs=4))

        # ---- constants / inputs ----
        # x comes in pre-transposed [CDIM, T]; one tile per (c, t-chunk) so
        # chunk-0 compute only waits on its own 6 DMAs.
        xTc = [
            big.tile([P, NCT * QW], DTM, tag=f"xT{t}", name=f"xT{t}")
            for t in range(NQC)
        ]
        # one DMA per (c, chunk): separate queues run in parallel (a single
        # merged DMA is one queue at ~47 GB/s and stalls the first GEMM)
        for t in range(NQC):
            for c in range(NCT):
                nc.sync.dma_start(
                    xTc[t][:, c * QW : (c + 1) * QW],
                    x_d[c * P : (c + 1) * P, t * QW : (t + 1) * QW],
                )

        def xT(c, t):
            return xTc[t][:, c * QW : (c + 1) * QW]
        wqk_t = const.tile([P, NCT * 3 * P], DTM, tag="wqk")
        wv_t = const.tile([P, NCT * HPC * DHEAD], DTM, tag="wv")
        for c in range(NCT):
            nc.scalar.dma_start(
                wqk_t[:, c * 3 * P : (c + 1) * 3 * P],
                wqk_d[c * P : (c + 1) * P, :],
            )
            nc.scalar.dma_start(
                wv_t[:, c * HPC * DHEAD : (c + 1) * HPC * DHEAD],
                wv_d[c * P : (c + 1) * P, :],
            )
        bias_t = const.tile([P, 3], DT, tag="bias")
        nc.scalar.dma_start(bias_t[:], b_d[:].rearrange("b p -> p b"))
        mask_t = const.tile([P, HPC * P], DTM, tag="tri01")
        nc.scalar.dma_start(mask_t[:], m_d[:])
        wp0 = const.tile([P, CDIM], DTM, tag="wp0")
        nc.scalar.dma_start(wp0[:], wp_d[0:P, :])
        wp1 = const.tile([64, CDIM], DTM, tag="wp1")
        nc.scalar.dma_start(wp1[:], wp_d[P : P + 64, :])
        ones_r = const.tile([65, 64], DTR, tag="ones_r")
        nc.scalar.dma_start(ones_r[64:65, :], onr_d[:])

        # V k-tiles [128, 3*65]: ones column preset once per tile.
        vk = [
            big.tile([P, 3 * 65], DTM, tag=f"vk{j}", name=f"vk{j}")
            for j in range(NTT)
        ]
        for j in range(NTT):
            nc.gpsimd.memset(
                vk[j][:].rearrange("p (h c) -> p h c", c=65)[:, :, 64:65], 1.0
            )

        # PSUM pools: S tri-tile 3 banks + psy 3 banks + misc 2 banks = 8.
        ps_s = ctx.enter_context(
            tc.tile_pool(name="ps_s", bufs=1, space=bass.MemorySpace.PSUM)
        )
        ps_y = ctx.enter_context(
            tc.tile_pool(name="ps_y", bufs=3, space=bass.MemorySpace.PSUM)
        )
        ps_m = ctx.enter_context(
            tc.tile_pool(name="ps_m", bufs=2, space=bass.MemorySpace.PSUM)
        )

        # qkv QK blocks (persist across chunks; written per chunk)
        blk = [big.tile([P, T], DTM, tag=f"blk{i}", name=f"blk{i}") for i in range(3)]
        q2dup = big.tile([P, T], DTM, tag="q2dup", name="q2dup")
        yT0 = big.tile([P, T], DTM, tag="yT0")  # rows: h0 | h1
        yT1 = big.tile([64, T], DTM, tag="yT1")  # h2

        def emit_proj(pqc):
            for qt in range(4 * pqc, 4 * pqc + 4):
                yield
                qsl = slice(qt * P, (qt + 1) * P)
                ost = work.tile([P, CDIM], DTM, tag="ost", bufs=3)
                for n in range(2):
                    nsl = slice(n * 384, (n + 1) * 384)
                    pso = ps_m.tile([P, 384], DT, tag="misc", bufs=2)
                    nc.tensor.matmul(
                        pso[:], yT0[:, qsl], wp0[:, nsl], start=True, stop=False
                    )
                    nc.tensor.matmul(
                        pso[:], yT1[:, qsl], wp1[:, nsl], start=False, stop=True
                    )
                    nc.vector.tensor_copy(ost[:, nsl], pso[:])
                nc.sync.dma_start(o_d[qsl, :], ost[:])

        def emit_qkv(tcn):
            cols = slice(tcn * QW, (tcn + 1) * QW)
            for bi in range(3):
                yield
                psq = ps_m.tile([P, QW], DT, tag="misc", bufs=2)
                for c in range(NCT):
                    nc.tensor.matmul(
                        psq[:],
                        wqk_t[:, c * 384 + bi * P : c * 384 + (bi + 1) * P],
                        xT(c, tcn),
                        start=(c == 0),
                        stop=(c == NCT - 1),
                    )
                nc.vector.tensor_scalar_add(
                    blk[bi][:, cols], psq[:], bias_t[:, bi : bi + 1]
                )
                if bi == 2:
                    # Q2 -> partitions 64-127 (cross-partition SBUF DMA)
                    nc.sync.dma_start(q2dup[64:P, cols], blk[2][0:64, cols])

        def emit_vdir(tcn):
            for j in range(4 * tcn, 4 * tcn + 4):
                yield
                toff = (j % 4) * P
                psv = ps_m.tile([P, HPC * DHEAD], DT, tag="misc", bufs=2)
                for c in range(NCT):
                    nc.tensor.matmul(
                        psv[:],
                        xT(c, tcn)[:, toff : toff + P],
                        wv_t[:, c * 192 : (c + 1) * 192],
                        start=(c == 0),
                        stop=(c == NCT - 1),
                    )
                nc.vector.tensor_copy(
                    vk[j][:].rearrange("p (h c) -> p h c", c=65)[:, :, 0:64],
                    psv[:].rearrange("p (h c) -> p h c", c=64),
                )

        # Q/K head slices for S^T matmuls:
        # h0: K0=blk1[0:64],  Q0=blk0[0:64]   -> row group 0
        # h1: K1=blk1[64:128],Q1=blk0[64:128] -> row group 64
        # h2: K2=blk2[64:128],Q2=q2dup[64:128]-> row group 64
        # Filler queue: generators yielding once per work item. During the
        # attention j-loop of chunk tc we drain one item per j-slot:
        # next chunk's qkv/V GEMMs and the previous chunk's projection.
        # This keeps the PE fed while act[j] blocks the S tri-tile.
        from collections import deque

        fillers = deque()

        def drain_one():
            while fillers:
                g = fillers[0]
                try:
                    next(g)
                    return
                except StopIteration:
                    fillers.popleft()

        def drain_all():
            while fillers:
                drain_one()

        # prologue: chunk 0 qkv + V
        for g in (emit_qkv(0), emit_vdir(0)):
            fillers.append(g)
        drain_all()

        for tcn in range(NQC):
            if tcn + 1 < NQC:
                fillers.append(emit_qkv(tcn + 1))
                fillers.append(emit_vdir(tcn + 1))
            # projections lag further so the last (largest) q-chunk's
            # attention spine still has PE filler work
            if tcn == 2:
                fillers.append(emit_proj(0))
            elif tcn == 3:
                fillers.append(emit_proj(1))
                fillers.append(emit_proj(2))

            # ---- attention for q-chunk qc=tcn (software-pipelined) ----
            qc = tcn
            nk = 4 * (qc + 1)
            psy = [
                ps_y.tile([65, QW], DT, tag=f"y{h}", bufs=1, name=f"psy{h}")
                for h in range(HPC)
            ]
            es_t = [None] * nk
            cs_of = [0 if (j - 4 * qc) < 1 else (j - 4 * qc) * P for j in range(nk)]

            def emit_s(j):
                cs = cs_of[j]
                ssl = slice(cs, QW)
                qsl_g = slice(qc * QW + cs, (qc + 1) * QW)
                jsl = slice(j * P, (j + 1) * P)
                pss = ps_s.tile([P, HPC * QW], DT, tag="s", bufs=1, name="pss")
                nc.tensor.matmul(
                    pss[:, 0 * QW + cs : 1 * QW],
                    blk[1][0:64, jsl],
                    blk[0][0:64, qsl_g],
                    start=True,
                    stop=True,
                )
                nc.tensor.matmul(
                    pss[:, 1 * QW + cs : 2 * QW],
                    blk[1][64:P, jsl],
                    blk[0][64:P, qsl_g],
                    start=True,
                    stop=True,
                )
                nc.tensor.matmul(
                    pss[:, 2 * QW + cs : 3 * QW],
                    blk[2][64:P, jsl],
                    q2dup[64:P, qsl_g],
                    start=True,
                    stop=True,
                )
                es = work.tile([P, HPC * QW], DTM, tag="es", bufs=4, name="es")
                es_t[j] = es
                nc.scalar.activation(
                    es[:].rearrange("p (h c) -> p h c", c=QW)[:, :, ssl],
                    pss[:].rearrange("p (h c) -> p h c", c=QW)[:, :, ssl],
                    EXP,
                    scale=float(SCALE),
                )
                if j - 4 * qc >= 0:
                    nc.gpsimd.tensor_mul(
                        es[:].rearrange("p (h c) -> p h c", c=QW)[:, :, cs : cs + P],
                        es[:].rearrange("p (h c) -> p h c", c=QW)[:, :, cs : cs + P],
                        mask_t[:].rearrange("p (h c) -> p h c", c=P),
                    )

            def emit_av(j):
                cs = cs_of[j]
                ssl = slice(cs, QW)
                for h in range(HPC):
                    nc.tensor.matmul(
                        psy[h][:, ssl],
                        vk[j][:, 65 * h : 65 * h + 65],
                        es_t[j][:, h * QW + cs : (h + 1) * QW],
                        start=(j == 0),
                        stop=(j == nk - 1),
                    )
                es_t[j] = None

            for jj in range(nk + 2):
                if jj < nk:
                    emit_s(jj)
                if jj - 2 >= 0:
                    emit_av(jj - 2)
                drain_one()

            # ---- softmax normalization for this q-chunk ----
            # stage-major so the three heads' copy->matmul->recip->mul
            # chains pipeline across DVE/PE instead of serializing
            qcols = slice(qc * QW, (qc + 1) * QW)
            lrow = work.tile([65, HPC * QW], DTR, tag="l", bufs=1)
            for h in range(HPC):
                nc.vector.tensor_copy(
                    lrow[64:65, h * QW : (h + 1) * QW], psy[h][64:65, :]
                )
            rbps = []
            for h in range(HPC):
                rbp = ps_m.tile([64, QW], DT, tag="misc", bufs=2)
                nc.tensor.matmul(
                    rbp[:],
                    ones_r[64:65, :],
                    lrow[64:65, h * QW : (h + 1) * QW],
                    start=True,
                    stop=True,
                )
                rbps.append(rbp)
            rbs = []
            for h in range(HPC):
                rb = work.tile([64, QW], DT, tag="rb", bufs=3)
                nc.vector.reciprocal_approx_fast(out=rb[:], in_=rbps[h][:])
                rbs.append(rb)
            nc.vector.tensor_mul(yT0[0:64, qcols], psy[0][0:64, :], rbs[0][:])
            tmp = work.tile([64, QW], DTM, tag="ytmp", bufs=2)
            nc.vector.tensor_mul(tmp[:], psy[1][0:64, :], rbs[1][:])
            nc.scalar.dma_start(yT0[64:P, qcols], tmp[:])
            nc.vector.tensor_mul(yT1[0:64, qcols], psy[2][0:64, :], rbs[2][:])

        drain_all()
        fillers.append(emit_proj(NQC - 1))
        drain_all()

    nc.compile()
    return nc


_NC_CACHE = None


def _get_nc():
    global _NC_CACHE
    if _NC_CACHE is None:
        _NC_CACHE = build_nc()
    return _NC_CACHE


def _host_inputs(x, w_attn, b_attn, w_proj):
    """Per-core input dicts. Core c = batch (c//4), head-group (c%4)."""
    import ml_dtypes

    npm = ml_dtypes.bfloat16
    x = np.ascontiguousarray(np.asarray(x, dtype=np.float32))
    w_attn = np.asarray(w_attn, dtype=np.float32)
    b_attn = np.asarray(b_attn, dtype=np.float32)
    w_proj = np.asarray(w_proj, dtype=np.float32)

    # 0/1 lower-triangular mask tile for the diagonal 128-block (p <= f keeps)
    pp, ff = np.meshgrid(np.arange(P), np.arange(P), indexing="ij")
    tri01 = (pp <= ff).astype(np.float32)

    in_maps = []
    for core in range(N_CORES):
        b, hg = divmod(core, 4)
        hs = 3 * hg  # first head of this core
        # column ranges in the 2304-wide qkv dim: q at 0, k at 768, v at 1536
        q0, k0, v0 = 64 * hs, CDIM + 64 * hs, 2 * CDIM + 64 * hs
        # blocks: b0=[Q0|Q1] b1=[K0|K1] b2=[Q2|K2]
        wqk = np.concatenate(
            [
                w_attn[:, q0 : q0 + 128],
                w_attn[:, k0 : k0 + 128],
                w_attn[:, q0 + 128 : q0 + 192],
                w_attn[:, k0 + 128 : k0 + 192],
            ],
            axis=1,
        )
        wv = w_attn[:, v0 : v0 + 192]
        bqk = np.zeros((3, P), dtype=np.float32)
        bqk[0] = b_attn[q0 : q0 + 128]
        bqk[1] = b_attn[k0 : k0 + 128]
        bqk[2, 0:64] = b_attn[q0 + 128 : q0 + 192]
        bqk[2, 64:128] = b_attn[k0 + 128 : k0 + 192]
        wproj = np.ascontiguousarray(w_proj[64 * hs : 64 * hs + 192, :])
        in_maps.append(
            {
                "x": np.ascontiguousarray(x[b].T.astype(npm)),
                "wqk": np.ascontiguousarray(wqk.astype(npm)),
                "wv": np.ascontiguousarray(wv.astype(npm)),
                "bqk": bqk,
                "wproj": wproj.astype(npm),
                "tri01": np.tile(tri01, (1, HPC)).astype(npm),
                "ones": np.ones((P, HPC), dtype=npm),
                "ones_r": np.ones((1, 64), dtype=np.float32),
            }
        )
    return in_maps


def run(x, w_attn, b_attn, w_proj, b_proj, trace=False):
    nc = _get_nc()
    in_maps = _host_inputs(x, w_attn, b_attn, w_proj)
    res = bass_utils.run_bass_kernel_spmd(
        nc, in_maps, core_ids=list(range(N_CORES)), trace=trace
    )
    B = 2
    b_attn = np.asarray(b_attn, dtype=np.float64)
    w_proj = np.asarray(w_proj, dtype=np.float64)
    out = np.zeros((B, T, CDIM), dtype=np.float64)
    for core in range(N_CORES):
        out[core // 4] += res.results[core]["out"].astype(np.float64)
    # V-bias folded out on device (softmax rows sum to 1): add b_v @ w_proj.
    b_v = b_attn[2 * CDIM : 3 * CDIM]
    out += (np.asarray(b_proj, dtype=np.float64) + b_v @ w_proj)[None, None, :]
    return out.astype(np.float32), res


def kernel(x, w_attn, b_attn, w_proj, b_proj):
    out, _ = run(x, w_attn, b_attn, w_proj, b_proj, trace=False)
    return out
